# revision 30
# baseline (speedup 1.0000x reference)
"""Linear attention ("Transformers are RNNs") on 8 Trainium2 NeuronCores.

Problem: N=8, L=S=8192, H=8, D=Dv=32, f32.
    phi(x) = elu(x)+1
    A[d,v] = sum_s phi(K)[s,d] V[s,v]     (the /v_length ... *v_length cancels)
    b[d]   = sum_s phi(K)[s,d]
    out[l,v] = (sum_d phi(Q)[l,d] A[d,v]) / (sum_d phi(Q)[l,d] b[d] + EPS)

Sharding: batch element n -> core n (fully independent, no collectives).

Design (v7) — single continuous DMA-bound stream, group-pipelined:
  - Heads split into G=2 groups of 4 (linear attention is separable per
    head).  K/V stream group-major, so group 0's A/b finish at the
    half-way point of the input stream and group 0's entire query pass
    (matmuls, reciprocal, normalize, output DMA) overlaps group 1's K/V
    accumulation.  Only group 1's query pass sits in the tail.
  - phi via the exact identity  phi(x) = min(exp(x), max(x+1, 1)).  The
    host ships x+1 (bf16); ACT computes exp((x+1)-1) via its bias input;
    DVE does tensor_scalar max (4x mode) + tensor_tensor min (2x mode).
    (A single scalar_tensor_tensor would be fewer ops but STT has no 2x
    uop and measures ~20% slower overall.)
  - All DMAs are large contiguous slabs: 8x K|V slab-pairs (1MB), 8x Q
    (512KB) on the sync queue in stream order; 16x out (256KB) on the
    gpsimd queue so a not-yet-ready output never head-of-line-blocks the
    input stream.  Measured input rate ~400-420 GB/s.
  - Normalize: one tensor_tensor per q-macro pair over a 2-bank PSUM
    tile [128, 1024] with a stride-0 broadcast reciprocal operand (the
    f32 PSUM read forces 1x mode anyway, so broadcast costs nothing).
  - reciprocal_approx_fast per 32 denominator columns (~18 correct
    bits; den ~1e5 so EPS=1e-6 is a 1e-11 perturbation and is dropped).
  - PE HAM clock gate: a junk-MM burst at kernel start plus two dummy
    matmuls written into each tail PSUM tile (immediately overwritten by
    the real numer matmuls) keep the PE at 8/8 (2.4 GHz) through the
    entire query tail — measured 8/8 for the full B-phase.
  - PSUM: 1 bank A/b accumulator (reused across groups), 3x 2-bank
    numer tiles, 1 den bank.  Engine busy (measured): DVE 53.8us (87%
    utilization over its active span), ACT ~38us, DMA ~47us, PE warm.

Measured on 8 NeuronCores: HW exec ~74.4-76.8us across runs (run-to-run
variance from HAM phase / shared HBM), rel err 2.55e-3 (gate 2e-2).
Lineage: v1 two-phase 90.3us -> v2 group-pipelined 85.0 -> v3 schedule
84.4 -> v4 (STT phi regression) 83.7 -> v5 packed-phi+buffers 78.1 ->
v6 og-split 81.0 -> v7 PSUM-junk warm tail 75.5 -> v8 merged qprep 74.4
(v9 coarse 4096-col phi tiles regressed to 78.2 and was reverted: big
DVE ops head-of-line-block the in-order queue and stall the normalize
cadence; 2048-col granularity is the sweet spot).  An ACT-assisted tail
normalize (ACT evacuates PSUM to bf16 + materializes the broadcast
reciprocal so the DVE multiply runs 2x — mechanism verified at 682ns vs
1226ns) was neutral overall because the tail is PE-queue-bound, not
DVE-bound; the code path remains (act_path=True) but is disabled.
Halving the tail warm-keeping junk matmuls loses the 8/8 clock gate and
regresses ~6us — two per pair is load-bearing.
v11 (current): kv_g1 streams BEFORE qq_g1 so the last kv slab (which
gates assemble(1) and the whole tail) arrives at ~34us instead of ~41;
qq_g1 then arrives during the tail with each phi(Q) computed just ahead
of its consumer pairs.  Output stream starts ~5.5us earlier; best
measured 73.9us.  Two refinements regressed and were reverted: placing
the first tail Q slab (DMA or compute) ahead of the last kv slab delays
the barrier either via the serial DMA queue or the in-order DVE queue.
An ACT-heavy tail phi (exp(-relu(-x)) chain, code at qprep(act2=True),
disabled) put 4.8us serial ACT chains on the tail dependency path and
regressed badly.  NOTE on measurement: sustained back-to-back benching
thermally throttles all engines to ~5/6 clock (uniform +19% op
durations, exec drifting 74->88us); a few minutes idle restores it.
Numbers here are cool-chip numbers.
"""

import sys

for _p in ("/opt/trn_rl_repo",):
    if _p not in sys.path:
        sys.path.insert(0, _p)

import ml_dtypes
import numpy as np

from concourse import bacc, bass, mybir, tile
from concourse.bass_utils import run_bass_kernel_spmd

# ---------------------------------------------------------------- constants
N_BATCH = 8
L = 8192
S = 8192
H = 8
D = 32
P = 128

F32 = mybir.dt.float32
BF16 = mybir.dt.bfloat16
AF = mybir.ActivationFunctionType
OP = mybir.AluOpType

G = 2          # head groups (4 heads each; 4*32 = 128 partitions)
NMP = 4        # K/V slab-pairs per group (2048 s-rows each)
MB = 16        # 128-row s-subtiles per slab-pair
VA = P + 1     # 129: V group columns + ones column
SLAB = 2056    # one old slab: 8*128 K cols + 8*129 V cols
KVCOLS = 2 * SLAB  # 4112
NDP = 4        # Q double-pairs per group (2048 l-columns each)
QCOLS = 2048


def _bcast_last(ap, n):
    """Append a stride-0 dim of size n to an AP (free-dim broadcast)."""
    ap = ap.unsqueeze(ap.ndim)
    return ap.broadcast_to(tuple(ap.shape[:-1]) + (n,))


def _build_body(nc, tc, qq, kv, og):
    with (
        tc.tile_pool(name="iokv", bufs=4) as iokv,
        tc.tile_pool(name="ioq", bufs=6) as ioq,
        tc.tile_pool(name="ewk", bufs=3) as ewk,
        tc.tile_pool(name="ewq", bufs=3) as ewq,
        tc.tile_pool(name="qp", bufs=1) as qp,
        tc.tile_pool(name="misc", bufs=1) as misc,
        tc.tile_pool(name="small", bufs=3) as small,
        tc.tile_pool(name="outp", bufs=3) as outp,
        tc.tile_pool(name="ev", bufs=2) as ev,
        tc.tile_pool(name="pacc", bufs=1, space="PSUM") as paccp,
        tc.tile_pool(name="psn", bufs=3, space="PSUM") as psn,
        tc.tile_pool(name="psd", bufs=1, space="PSUM") as psd,
    ):
        pacc = {}
        phiq = {}
        amat = {}
        bmat = {}

        # bias columns for exp((x+1) - 1) and relu(-(x+1) + 1)
        nbias = misc.tile([P, 1], F32, tag="nbias", name="nbias")
        nc.gpsimd.memset(nbias[:], -1.0)
        pbias = misc.tile([P, 1], F32, tag="pbias", name="pbias")
        nc.gpsimd.memset(pbias[:], 1.0)

        # HAM warm-up: dense dummy matmuls while the first DMAs prefill.
        wz = misc.tile([P, 512], BF16, tag="warm", name="warm")
        nc.gpsimd.memset(wz[:], 0.0)
        pacc[0] = paccp.tile([P, 512], F32, tag="pacc", name="pacc")
        for _ in range(9):
            nc.tensor.matmul(
                pacc[0][:], wz[:, 0:P], wz[:], start=True, stop=True
            )

        def a_macro(g, mp2):
            """One K|V slab-pair (2048 s-rows) of group g."""
            if mp2 == 0 and g > 0:
                pacc[g] = paccp.tile([P, 512], F32, tag="pacc", name="pacc")
            kvt = iokv.tile([P, KVCOLS], BF16, tag="kv")
            split = g == 0 and mp2 == 0
            if split:
                hs = SLAB // 2  # 1028
                for c0 in range(0, KVCOLS, hs):
                    nc.sync.dma_start(
                        kvt[:, c0 : c0 + hs], kv[g, mp2][:, c0 : c0 + hs]
                    )
            else:
                nc.sync.dma_start(kvt[:], kv[g, mp2])
            # K+1 part: two 1024-col runs at offsets 0 and SLAB
            kp1 = kvt[:].rearrange("p (s c) -> p s c", s=2, c=SLAB)[:, :, 0:1024]
            e = ewk.tile([P, 2048], BF16, tag="ke")
            ph = ewk.tile([P, 2048], BF16, tag="kphi")
            t = ewk.tile([P, 2048], BF16, tag="kt")
            e2 = e[:].rearrange("p (s c) -> p s c", s=2)
            t2 = t[:].rearrange("p (s c) -> p s c", s=2)
            ph2 = ph[:].rearrange("p (s c) -> p s c", s=2)
            # e = exp((x+1) - 1);  t = max(x+1, 1);  phi = min(e, t)
            halves = 2 if split else 1
            for hh in range(halves):
                sl = slice(hh, None) if halves == 1 else slice(hh, hh + 1)
                nc.scalar.activation(e2[:, sl], kp1[:, sl], AF.Exp, bias=nbias[:])
                nc.vector.tensor_scalar(t2[:, sl], kp1[:, sl], 1.0, None, OP.max)
                nc.vector.tensor_tensor(ph2[:, sl], e2[:, sl], t2[:, sl], OP.min)
            first = mp2 == 0
            last = mp2 == NMP - 1
            for b in range(MB):
                voff = (b // 8) * SLAB + 1024 + (b % 8) * VA
                nc.tensor.matmul(
                    pacc[g][:, 0:VA],
                    ph[:, b * P : (b + 1) * P],
                    kvt[:, voff : voff + VA],
                    start=(first and b == 0),
                    stop=(last and b == MB - 1),
                )

        def qprep(g, dp, act2=False):
            qt = ioq.tile([P, QCOLS], BF16, tag="qt")
            nc.sync.dma_start(qt[:], qq[g, dp])
            e = ewq.tile([P, QCOLS], BF16, tag="qe")
            ph = qp.tile([P, QCOLS], BF16, tag=f"phiq{g}_{dp}",
                         name=f"phiq{g}_{dp}")
            if act2:
                # phi = max(exp(-relu(-x)), x+1): two ACT ops + one DVE op.
                # Used where ACT is idle and the DVE conveyor is the pacer.
                u = ewq.tile([P, QCOLS], BF16, tag="qu", name="qu")
                nc.scalar.activation(u[:], qt[:], AF.Relu,
                                     bias=pbias[:], scale=-1.0)
                nc.scalar.activation(e[:], u[:], AF.Exp, scale=-1.0)
                nc.vector.tensor_tensor(ph[:], e[:], qt[:], OP.max)
            else:
                t = ewq.tile([P, QCOLS], BF16, tag="qt2")
                nc.scalar.activation(e[:], qt[:], AF.Exp, bias=nbias[:])
                nc.vector.tensor_scalar(t[:], qt[:], 1.0, None, OP.max)
                nc.vector.tensor_tensor(ph[:], e[:], t[:], OP.min)
            phiq[(g, dp)] = ph

        def assemble(g):
            am = misc.tile([P, P], BF16, tag=f"am{g}", name=f"am{g}")
            bm = misc.tile([P, 4], BF16, tag=f"bm{g}", name=f"bm{g}")
            nc.vector.memset(am[:], 0.0)
            nc.vector.memset(bm[:], 0.0)
            for j in range(4):
                r0 = 32 * j
                nc.scalar.copy(
                    am[r0 : r0 + 32, r0 : r0 + 32],
                    pacc[g][r0 : r0 + 32, r0 : r0 + 32],
                )
                nc.scalar.copy(
                    bm[r0 : r0 + 32, j : j + 1],
                    pacc[g][r0 : r0 + 32, P : P + 1],
                )
            amat[g] = am
            bmat[g] = bm

        # state shared across a double-pair (two b_pair calls)
        dpstate = {}

        def b_pair(g, mp, act_path=False):
            """Query pass for one pair of q-macros (1024 l-rows)."""
            half = mp % 2
            if half == 0:
                dpstate["dn"] = psd.tile([P, 64], F32, tag="dn", name="dn")
                dpstate["ot"] = outp.tile([P, 2 * 1024], BF16, tag="ot", name="ot")
                dpstate["rcp"] = small.tile([P, 64], F32, tag="rcp", name="rcp")
            dn = dpstate["dn"]
            ot = dpstate["ot"]
            rcp = dpstate["rcp"]
            nm = psn.tile([P, 1024], F32, tag="nm")
            if g == 1:
                # dummy matmuls, fully overwritten by the real ones below:
                # they keep the PE activity monitor at 8/8 across norm waits
                for _ in range(2):
                    nc.tensor.matmul(
                        nm[:, 0:512], wz[:, 0:P], wz[:], start=True, stop=True
                    )
            ph = phiq[(g, mp // 2)]
            for qs in range(8):  # (qmacro-in-pair, subtile)
                w = ph[:, (half * 8 + qs) * P : (half * 8 + qs + 1) * P]
                nc.tensor.matmul(
                    nm[:, qs * P : (qs + 1) * P], w, amat[g][:],
                    start=True, stop=True,
                )
                nc.tensor.matmul(
                    dn[:, half * 32 + qs * 4 : half * 32 + (qs + 1) * 4],
                    w, bmat[g][:], start=True, stop=True,
                )
            nc.vector.reciprocal_approx_fast(
                out=rcp[:, half * 32 : half * 32 + 32],
                in_=dn[:, half * 32 : half * 32 + 32],
            )
            osl = ot[:, half * 1024 : (half + 1) * 1024]
            rsl = rcp[:, half * 32 : half * 32 + 32]
            if act_path:
                # ACT (idle in the tail) evacuates the PSUM quad to bf16 and
                # materializes the broadcast reciprocal as a step-1 tensor so
                # the DVE multiply runs in 2x packed mode.
                nmb = ev.tile([P, 1024], BF16, tag="nmb", name="nmb")
                rb = ev.tile([P, 1024], BF16, tag="rb", name="rb")
                nc.scalar.copy(nmb[:], nm[:])
                nc.scalar.copy(
                    rb[:].rearrange("p (qs j c) -> p qs j c", qs=8, j=4, c=32),
                    _bcast_last(
                        rsl.rearrange("p (qs j) -> p qs j", qs=8, j=4), 32
                    ),
                )
                nc.vector.tensor_tensor(osl, nmb[:], rb[:], OP.mult)
                nc.gpsimd.dma_start(og[g, mp], osl)
            elif False:
                # final pair: split the normalize so the last output DMA
                # starts earlier (shorter drain)
                for hh in range(2):
                    hsl = slice(hh * 512, (hh + 1) * 512)
                    nc.vector.tensor_tensor(
                        osl[:, hsl].rearrange(
                            "p (qs j c) -> p qs j c", qs=4, j=4, c=32
                        ),
                        nm[:, hsl].rearrange(
                            "p (qs j c) -> p qs j c", qs=4, j=4, c=32
                        ),
                        _bcast_last(
                            rsl[:, hh * 16 : (hh + 1) * 16].rearrange(
                                "p (qs j) -> p qs j", qs=4, j=4
                            ),
                            32,
                        ),
                        OP.mult,
                    )
                    nc.gpsimd.dma_start(og[g, mp][:, hsl], osl[:, hsl])
            else:
                nc.vector.tensor_tensor(
                    osl.rearrange("p (qs j c) -> p qs j c", qs=8, j=4, c=32),
                    nm[:].rearrange("p (qs j c) -> p qs j c", qs=8, j=4, c=32),
                    _bcast_last(
                        rsl.rearrange("p (qs j) -> p qs j", qs=8, j=4), 32
                    ),
                    OP.mult,
                )
                nc.gpsimd.dma_start(og[g, mp], osl)

        # -------- group 0: A/b accumulation + group 0 Q prep ----------------
        for mp2 in range(NMP):
            a_macro(0, mp2)
            qprep(0, mp2)
        assemble(0)

        # -------- group 1 accumulation overlapped with group 0 queries ------
        # (group 1 qprep interleaved here so group 0's normalizes are not
        #  stuck behind it in the in-order DVE queue)
        # kv_g1 streams first (it gates assemble(1) and the whole tail);
        # qq_g1 arrives during the tail and each phi(Q) is computed just
        # before its consumer pairs so the in-order queues never block.
        for mp2 in range(NMP):
            a_macro(1, mp2)
            b_pair(0, 2 * mp2)
            b_pair(0, 2 * mp2 + 1)
        assemble(1)

        # ---------------- group 1 queries (tail) ----------------
        # the back half is a DVE-bound conveyor (norms+recips+phi(Q));
        # route the middle tail pairs' normalize through the idle ACT
        # (evacuate PSUM to bf16 + materialize the reciprocal broadcast)
        # so the DVE multiply runs in 2x packed mode.  First pair stays
        # direct (barrier-adjacent latency), last pair stays direct
        # (shortest path to the final output DMA).
        for dp in range(NDP):
            qprep(1, dp)
            b_pair(1, 2 * dp, act_path=(dp > 0))
            b_pair(1, 2 * dp + 1, act_path=(dp < NDP - 1))


_NC_CACHE = None


def build_nc():
    global _NC_CACHE
    if _NC_CACHE is not None:
        return _NC_CACHE
    nc = bacc.Bacc(
        "TRN2",
        target_bir_lowering=False,
        debug=False,
        enable_asserts=False,
        num_devices=N_BATCH,
    )
    qq = nc.dram_tensor("qq", [G, NDP, P, QCOLS], BF16, kind="ExternalInput").ap()
    kv = nc.dram_tensor("kv", [G, NMP, P, KVCOLS], BF16, kind="ExternalInput").ap()
    og = nc.dram_tensor("og", [G, 2 * NDP, P, 1024], BF16, kind="ExternalOutput").ap()
    with tile.TileContext(nc) as tc:
        _build_body(nc, tc, qq, kv, og)
    nc.compile()
    _NC_CACHE = nc
    return nc


def make_in_maps(queries, keys, values):
    queries = np.asarray(queries, dtype=np.float32)
    keys = np.asarray(keys, dtype=np.float32)
    values = np.asarray(values, dtype=np.float32)
    bf = ml_dtypes.bfloat16
    in_maps = []
    for n in range(N_BATCH):
        kvn = np.empty((G, 8, P, SLAB), dtype=bf)
        qqn = np.empty((G, NDP, P, QCOLS), dtype=bf)
        for g in range(G):
            # K group slab (shifted by +1 for the bias-exp trick)
            Kg = keys[n][:, 4 * g : 4 * g + 4, :].reshape(S, P) + 1.0
            kvn[g, :, :, 0:1024] = (
                Kg.reshape(8, 8, P, P).transpose(0, 2, 1, 3)
                .reshape(8, P, 1024).astype(bf)
            )
            # V group slab with ones column
            Vg = values[n][:, 4 * g : 4 * g + 4, :].reshape(S, P)
            V1 = np.ones((S, VA), dtype=np.float32)
            V1[:, 0:P] = Vg
            kvn[g, :, :, 1024:] = (
                V1.reshape(8, 8, P, VA).transpose(0, 2, 1, 3)
                .reshape(8, P, 8 * VA).astype(bf)
            )
            # Q+1 transposed group-major: [dp][jd, l]
            Qg = queries[n][:, 4 * g : 4 * g + 4, :].reshape(L, P) + 1.0
            qqn[g] = (
                Qg.T.reshape(P, NDP, QCOLS).transpose(1, 0, 2).astype(bf)
            )
        # pair adjacent slabs: [g, 4, p, 2*SLAB]
        kvp = np.ascontiguousarray(
            kvn.reshape(G, NMP, 2, P, SLAB).transpose(0, 1, 3, 2, 4)
            .reshape(G, NMP, P, KVCOLS)
        )
        in_maps.append({"qq": qqn, "kv": kvp})
    return in_maps


def run(queries, keys, values, trace=False, **kwargs):
    nc = build_nc()
    in_maps = make_in_maps(queries, keys, values)
    res = run_bass_kernel_spmd(
        nc, in_maps, core_ids=list(range(N_BATCH)), trace=trace, **kwargs
    )
    outs = []
    for n in range(N_BATCH):
        o = res.results[n]["og"].astype(np.float32)
        # og[g, mp, p, (q, s, j, v)]; l = ((mp*2+q)*4+s)*128+p
        o = o.reshape(G, 2 * NDP, P, 2, 4, 4, 32)
        o = o.transpose(1, 3, 4, 2, 0, 5, 6).reshape(L, H, D)
        outs.append(o)
    return np.stack(outs, axis=0), res


def kernel(queries, keys, values):
    out, _ = run(queries, keys, values, trace=False)
    return out
